# revision 17
# baseline (speedup 1.0000x reference)
"""Trainium2 Bass kernel for the BEMv13 MoE-LoRA module.

Computation (per token t, full problem):
  base  = x @ W_base.T + b_base
  w     = softmax(x @ W_router + b_router)        # E=2 experts
  H     = x @ A_cat.T                             # [T, 16] LoRA down-proj, both experts
  G     = H * w_broadcast * (alpha/rank)          # per-expert routing weight
  out   = base + G @ B_cat.T

Sharding: tokens (batch*seq = 16384) split evenly across 8 NeuronCores;
all weights replicated. No cross-core communication.

On-core algorithm (per core, 2048 tokens):
  - W_base is pre-transposed on host to W^T [D, O]; rounded on-chip to
    float32r (TF32-like) and kept resident in SBUF (128 KB/partition).
  - x arrives token-major; each [128,128] tile is transposed on the PE
    (fp32 transpose) and rounded to float32r during the PSUM->SBUF drain.
  - Main matmul: out[128 tok, 512 o] accumulated over 16 k-tiles in PSUM,
    float32r operands (1 cycle/row, ~fp32 dynamic range, ~1.3e-4 rel err).
  - Router logits difference and LoRA H are fused into one small rhs
    (aat, [D, 18]) sharing the same stationary x^T tiles.
  - softmax over 2 experts == sigmoid of the logit difference.
  - G^T (PE transpose of the scaled H) feeds a final K=16 accumulation
    step with B_cat^T, so the LoRA up-proj lands in the same PSUM banks.
"""

import numpy as np

P = 128
D = 2048
O = 2048
KT = D // P            # 16 k-tiles
TOK = 2048             # tokens per core
NSTR = TOK // 256      # 8 stripes of 256 tokens (2 tok-tiles)
HN = 18                # 16 LoRA cols + 1 router-diff col + 1 pad (fp32r needs even N)
ER = 16                # E*R
SCALE = 16.0 / 8.0
NCORES = 8

_CACHE = {}


def _build():
    import concourse.tile as tile
    import concourse.masks as masks
    from concourse import bacc, mybir

    f32 = mybir.dt.float32
    f32r = mybir.dt.float32r

    nc = bacc.Bacc("TRN2", target_bir_lowering=False, debug=False)

    xs_d = nc.dram_tensor("xs", [TOK, D], f32, kind="ExternalInput")
    wt_d = nc.dram_tensor("wt", [D, O], f32, kind="ExternalInput")
    aat_d = nc.dram_tensor("aat", [P, KT * HN], f32, kind="ExternalInput")
    bt_d = nc.dram_tensor("bt", [ER, O], f32, kind="ExternalInput")
    bb_d = nc.dram_tensor("bb", [1, O], f32, kind="ExternalInput")
    brd_d = nc.dram_tensor("brd", [1, 1], f32, kind="ExternalInput")
    out_d = nc.dram_tensor("out", [TOK, O], f32, kind="ExternalOutput")

    with tile.TileContext(nc) as tc:
        with (
            tc.tile_pool(name="res", bufs=1) as res,
            tc.tile_pool(name="big2k", bufs=2) as big2k,
            tc.tile_pool(name="xpool", bufs=2) as xpool,
            tc.tile_pool(name="xtpool", bufs=8) as xtpool,
            tc.tile_pool(name="small", bufs=2) as small,
            tc.tile_pool(name="psA", bufs=5, space="PSUM") as psA,
            tc.tile_pool(name="psT", bufs=2, space="PSUM") as psT,
            tc.tile_pool(name="psH", bufs=1, space="PSUM") as psH,
        ):
            ident = res.tile([P, P], f32, tag="ident")
            masks.make_identity(nc, ident[:])

            # x tok-tile loads go on the ACT HWDGE queue set, W^T streams on
            # the SP set — independent queues, PE transposes start early.
            NT = TOK // P
            x32_tiles = [None] * NT

            def load_x(t, chunks=1):
                x32_tiles[t] = xpool.tile([P, D], f32, tag="x32", name=f"x32_{t}")
                cw = D // chunks
                for cc in range(chunks):
                    nc.scalar.dma_start(
                        x32_tiles[t][:, cc * cw:(cc + 1) * cw],
                        xs_d[t * P:(t + 1) * P, cc * cw:(cc + 1) * cw])

            load_x(0, chunks=4)
            load_x(1, chunks=2)

            # small constants also on the ACT queue set (land in ~us)
            aat32 = res.tile([P, KT * HN], f32, tag="aat32")
            nc.scalar.dma_start(aat32[:], aat_d[:])
            bt32 = res.tile([ER, O], f32, tag="bt32")
            nc.scalar.dma_start(bt32[:], bt_d[:])
            # biases DMA'd directly with partition-broadcast source APs
            bb128 = res.tile([P, O], f32, tag="bb128")
            nc.gpsimd.dma_start(bb128[:], bb_d[:].broadcast_to((P, O)))
            brd128 = res.tile([P, 1], f32, tag="brd128")
            nc.gpsimd.dma_start(brd128[:], brd_d[:].broadcast_to((P, 1)))

            # --- W^T DMAs: SP queue, half-slabs so the arrival granularity
            # (~1.5us) matches the PE's per-k consumption rate.
            HO = O // 2
            wt_r = res.tile([P, KT * O], f32r, tag="wt_r")
            w32s = []
            for k in range(KT):
                w32 = big2k.tile([P, O], f32, tag="big2k", name=f"w32_{k}")
                for hh in range(2):
                    nc.sync.dma_start(w32[:, hh * HO:(hh + 1) * HO],
                                      wt_d[k * P:(k + 1) * P, hh * HO:(hh + 1) * HO])
                w32s.append(w32)

            # DVE cast order: first two W slabs first (they gate the first
            # matmuls), then the small constants, then the rest of W.
            def wcast(k):
                for hh in range(2):
                    nc.vector.tensor_copy(
                        wt_r[:, k * O + hh * HO:k * O + (hh + 1) * HO],
                        w32s[k][:, hh * HO:(hh + 1) * HO])

            wcast(0)
            wcast(1)
            aat_r = res.tile([P, KT * HN], f32r, tag="aat_r")
            nc.vector.tensor_copy(aat_r[:], aat32[:])
            identr = res.tile([P, P], f32r, tag="identr")
            nc.vector.tensor_copy(identr[:], ident[:])
            bt_r = res.tile([ER, O], f32r, tag="bt_r")
            nc.vector.tensor_copy(bt_r[:], bt32[:])
            for k in range(2, KT):
                wcast(k)


            # --- main loop: one 128-token tile at a time.
            # Tile t's body also emits tile t+1's PE transposes, interleaved
            # after each 4-k matmul chunk, so the PE stays dense even while
            # trailing the W^T DMA stream at startup.
            xtg_all = [[None] * 4 for _ in range(NT)]

            def emit_transpose_group(t, c):
                st = psT.tile([P, 512], f32, tag="tstage", name=f"ts_{t}_{c}")
                for q in range(4):
                    nc.tensor.transpose(
                        st[:, q * P:(q + 1) * P],
                        x32_tiles[t][:, (c * 4 + q) * P:(c * 4 + q + 1) * P],
                        ident[:],
                    )
                xt = xtpool.tile([P, 512], f32r, tag="xt", name=f"xt_{t}_{c}")
                nc.scalar.copy(xt[:], st[:])
                xtg_all[t][c] = xt

            for c in range(4):
                emit_transpose_group(0, c)
            for c in range(4):
                emit_transpose_group(1, c)

            for t in range(NT):
                if t + 1 < NT and x32_tiles[t + 1] is None:
                    load_x(t + 1)

                accs = [
                    psA.tile([P, 512], f32, tag="acc", name=f"acc_{t}_{j}")
                    for j in range(4)
                ]
                h = psH.tile([P, HN], f32, tag="h", name=f"h_{t}")
                for k in range(KT):
                    lhs = xtg_all[t][k // 4][:, (k % 4) * P:(k % 4 + 1) * P]
                    nc.tensor.matmul(h[:], lhs, aat_r[:, k * HN:(k + 1) * HN],
                                     start=(k == 0), stop=(k == KT - 1))
                    for j in range(4):
                        nc.tensor.matmul(
                            accs[j][:], lhs,
                            wt_r[:, k * O + j * 512:k * O + (j + 1) * 512],
                            start=(k == 0), stop=False,
                        )
                    if k % 4 == 3 and t >= 1 and t + 1 < NT:
                        emit_transpose_group(t + 1, k // 4)
                    # HAM warmers: during the W^T-trailing phase (tiles 0-1)
                    # the PE idles between k-groups and re-throttles to half
                    # clock; re-reading the just-arrived slab keeps it warm.
                    if t == 0 or (t == 1 and k < 12):
                        wdum = psT.tile([P, 512], f32, tag="tstage", name=f"wd_{t}_{k}")
                        nc.tensor.matmul(wdum[:], lhs, wt_r[:, k * O:k * O + 512],
                                         start=True, stop=True)

                # routing: w1 = sigmoid(dlogit + brd); scaled by alpha/rank
                srow = small.tile([P, 1], f32, tag="srow", name=f"srow_{t}")
                nc.scalar.activation(srow[:], h[:, ER:ER + 1],
                                     mybir.ActivationFunctionType.Sigmoid,
                                     bias=brd128[:, 0:1], scale=1.0)
                w1s = small.tile([P, 1], f32, tag="w1s", name=f"w1s_{t}")
                nc.vector.tensor_scalar_mul(w1s[:], srow[:], SCALE)
                w0s = small.tile([P, 1], f32, tag="w0s", name=f"w0s_{t}")
                nc.vector.tensor_scalar(w0s[:], srow[:], -SCALE, SCALE,
                                        mybir.AluOpType.mult, mybir.AluOpType.add)
                g = small.tile([P, ER], f32r, tag="g", name=f"g_{t}")
                nc.vector.tensor_scalar_mul(g[:, 0:8], h[:, 0:8], w0s[:])
                nc.vector.tensor_scalar_mul(g[:, 8:16], h[:, 8:16], w1s[:])

                gst = psT.tile([ER, P], f32r, tag="tstage", name=f"gst_{t}")
                nc.tensor.transpose(gst[:], g[:], identr[:])
                gt = small.tile([ER, P], f32r, tag="gt", name=f"gt_{t}")
                nc.vector.tensor_copy(gt[:], gst[:])

                for j in range(4):
                    nc.tensor.matmul(accs[j][:], gt[:],
                                     bt_r[:, j * 512:(j + 1) * 512],
                                     start=False, stop=True)

                outt = big2k.tile([P, O], f32, tag="big2k", name=f"out_{t}")
                for j in range(4):
                    nc.vector.tensor_add(outt[:, j * 512:(j + 1) * 512],
                                         accs[j][:], bb128[:, j * 512:(j + 1) * 512])
                nc.sync.dma_start(out_d[t * P:(t + 1) * P, :], outt[:])

    nc.compile()
    return nc


def _prep_host(x, W_base, b_base, A, B, W_router, b_router):
    """Host-side layout prep + sharding. Returns per-core input maps."""
    x_flat = np.ascontiguousarray(x, dtype=np.float32).reshape(-1, D)
    wt = np.ascontiguousarray(W_base.T, dtype=np.float32)           # [D, O]
    a_cat = np.asarray(A, dtype=np.float32).reshape(ER, D)          # [16, D]
    aat = np.zeros((D, HN), dtype=np.float32)
    aat[:, :ER] = a_cat.T
    aat[:, ER] = np.asarray(W_router, dtype=np.float32)[:, 1] - np.asarray(W_router, dtype=np.float32)[:, 0]
    # pre-arrange for contiguous per-partition DMA: [P, KT*HN]
    aat = np.ascontiguousarray(aat.reshape(KT, P, HN).transpose(1, 0, 2).reshape(P, KT * HN))
    b_cat = np.concatenate([np.asarray(B, dtype=np.float32)[0],
                            np.asarray(B, dtype=np.float32)[1]], axis=1)  # [O, 16]
    bt = np.ascontiguousarray(b_cat.T)                               # [16, O]
    bb = np.asarray(b_base, dtype=np.float32).reshape(1, O)
    brd = np.array([[np.float32(b_router[1]) - np.float32(b_router[0])]], dtype=np.float32)

    in_maps = []
    for c in range(NCORES):
        in_maps.append({
            "xs": x_flat[c * TOK:(c + 1) * TOK],
            "wt": wt,
            "aat": aat,
            "bt": bt,
            "bb": bb,
            "brd": brd,
        })
    return in_maps


def _enable_ldw_opt():
    """Compile this kernel with walrus's LDWEIGHTS dedup pass. Consecutive
    matmuls here share one stationary operand per k-group; the dedup removes
    the redundant reloads (validated bit-identical output vs. the default)."""
    if _CACHE.get("ldw_patched"):
        return
    import concourse.bass_utils as bu

    orig = bu.run_command

    def patched(argv, **kw):
        argv = [a.replace("--enable-ldw-opt=false", "--enable-ldw-opt=true")
                if isinstance(a, str) else a for a in argv]
        return orig(argv, **kw)

    bu.run_command = patched
    _CACHE["ldw_patched"] = True


def kernel(x, W_base, b_base, A, B, W_router, b_router):
    from concourse import bass_utils

    _enable_ldw_opt()
    if "nc" not in _CACHE:
        _CACHE["nc"] = _build()
    nc = _CACHE["nc"]

    in_maps = _prep_host(x, W_base, b_base, A, B, W_router, b_router)
    res = bass_utils.run_bass_kernel_spmd(nc, in_maps, core_ids=list(range(NCORES)))
    out = np.concatenate([res.results[c]["out"] for c in range(NCORES)], axis=0)
    return out.reshape(np.asarray(x).shape[0], -1, O)
